# revision 5
# baseline (speedup 1.0000x reference)
"""Trainium2 Bass kernel for BotNet-style sparse attention (4 heads, 64x64 map,
dh=128, decomposed 2D relative position bias).

Sharding: 8 cores = 4 heads x 2 query-halves. Each core computes its head's
q/k/v from the full fmap, builds the rel-pos bias row tensors on chip, and runs
flash-style attention in "transposed sim" orientation (keys on partitions,
queries on free dim) so no attention-matrix transposes are needed:

  simT[k, q] = K^T.T @ Q^T  (+ bias via indicator-matmul accumulation)
  expT = exp(SCALE * simT - 4)           (ACT, PSUM->SBUF fp16)
  outT[d, q] = sum_k V[k, d] * expT[k,q] (PSUM accumulation over key chunks)
  rowsum via DVE accumulate + ones-matmul partition reduce
  out = outT * (1/rowsum) broadcast      (K=1 outer-product matmul broadcast)

The rel-pos bias decomposes per query q=(hq,wq), key k=(hk,wk) as
  bias = Rh[q, hk-hq+63] + Rw[q, wk-wq+63]
The row tensors BT are built from two batched matmuls producing every rel
window at once in PSUM, evacuated whole to SBUF, sheared into per-query
diagonal bands with partition-offset SBUF->SBUF DMAs, and converted to bf16
by single big DVE copies. The bias then folds into sim via one accumulating
matmul against a 0/1 indicator matrix per key chunk.

Per-core inputs are key-permuted (own query half first) so the SPMD graph is
identical across cores; all per-core differences live in the input data.
"""

import numpy as np
import ml_dtypes

C, H, W = 512, 64, 64
HEADS, DH = 4, 128
L = H * W           # 4096
NQ = L // 2         # 2048 queries per core
QB = 1024           # query block
SCALE = DH ** -0.5
NCORES = 8

_GRAPH = None


def _build_graph():
    from concourse import bacc
    import concourse.mybir as mybir
    import concourse.tile as tile

    f32 = mybir.dt.float32
    bf16 = mybir.dt.bfloat16
    fp16 = mybir.dt.float16
    EXPF = mybir.ActivationFunctionType.Exp

    nc = bacc.Bacc(None)

    fmap_p = nc.declare_dram_parameter("fmapc", [16 * 128, 1024], bf16, isOutput=False)
    wt_p = nc.declare_dram_parameter("wt", [C, 384], bf16, isOutput=False)
    relh_p = nc.declare_dram_parameter("relh", [128, 96], bf16, isOutput=False)
    relw_p = nc.declare_dram_parameter("relw", [128, 127], bf16, isOutput=False)
    ind_p = nc.declare_dram_parameter("ind", [4 * 128, 1024], bf16, isOutput=False)
    onesh_p = nc.declare_dram_parameter("onesh", [128, 128], fp16, isOutput=False)
    bias4_p = nc.declare_dram_parameter("bias4", [128, 1], f32, isOutput=False)
    out_p = nc.declare_dram_parameter("out", [128, NQ], f32, isOutput=True)

    with tile.TileContext(nc) as tc:
        with tc.tile_pool(name="const", bufs=1) as cpool, \
             tc.tile_pool(name="big", bufs=1) as big, \
             tc.tile_pool(name="work", bufs=2) as work:

            # warm tile memset first in the gpsimd stream so PE warmup
            # matmuls can start right after the init barrier
            warm_sb = work.tile([128, 512], bf16, name="warm_sb", tag="warm")
            nc.gpsimd.memset(warm_sb, 0.0)

            # ---- constants to SBUF (small; on the slow SWDGE queue) ----
            relh_sb = cpool.tile([128, 96], bf16, name="relh_sb")
            relw_sb = cpool.tile([128, 127], bf16, name="relw_sb")
            ind_sb = cpool.tile([128, L], bf16, name="ind_sb")
            onesh_sb = cpool.tile([128, 128], fp16, name="onesh_sb")
            bias4_sb = cpool.tile([128, 1], f32, name="bias4_sb")

            # ---- weights first (small, unblock qkv matmuls), then fmap
            # t-major so each 1024-column stripe completes across all four
            # c-tiles early; spread across engine DMA queues for bandwidth ----
            F4 = [big.tile([128, L], bf16, name=f"F{c}") for c in range(4)]
            W4 = []
            w_engs = [nc.sync, nc.scalar, nc.sync, nc.scalar]
            for c in range(4):
                w = big.tile([128, 384], bf16, name=f"W{c}")
                w_engs[c].dma_start(out=w, in_=wt_p[c * 128:(c + 1) * 128, :])
                W4.append(w)
            dma_engs = [nc.sync, nc.scalar, nc.scalar, nc.sync]
            def fblk(c, t):
                b = c * 4 + t
                return fmap_p[b * 128:(b + 1) * 128, :]

            for h in range(2):
                for c in range(4):
                    dma_engs[c].dma_start(
                        out=F4[c][:, h * 512:(h + 1) * 512],
                        in_=fblk(c, 0)[:, h * 512:(h + 1) * 512])
            for t in range(1, 4):
                for c in range(4):
                    dma_engs[c].dma_start(
                        out=F4[c][:, t * 1024:(t + 1) * 1024], in_=fblk(c, t))
            for k in range(4):
                (nc.sync if k % 2 == 0 else nc.scalar).dma_start(
                    out=ind_sb[:, k * 1024:(k + 1) * 1024],
                    in_=ind_p[k * 128:(k + 1) * 128, :])
            nc.gpsimd.dma_start(out=relh_sb, in_=relh_p[:, :])
            nc.gpsimd.dma_start(out=relw_sb, in_=relw_p[:, :])
            nc.gpsimd.dma_start(out=onesh_sb, in_=onesh_p[:, :])
            nc.gpsimd.dma_start(out=bias4_sb, in_=bias4_p[:, :])

            QT = big.tile([128, NQ], bf16, name="QT")
            KT = big.tile([128, L], bf16, name="KT")
            VTt = big.tile([128, L], bf16, name="VTt")
            Vn = big.tile([128, L], bf16, name="Vn")
            BT = big.tile([128, NQ], bf16, name="BT")
            bh_stage = big.tile([96, NQ], f32, name="bh_stage")
            bw_stage = big.tile([127, NQ], f32, name="bw_stage")
            bh2 = big.tile([64, NQ], f32, name="bh2")
            bw2 = big.tile([64, NQ], f32, name="bw2")

            # ---- PE warmup: dummy matmuls on the memset tile fill the early
            # DMA wait and open the HAM clock-gate before real matmuls ----
            with tc.tile_pool(name="psW", bufs=1, space="PSUM") as psW:
                wps = psW.tile([128, 512], f32, name="warm_ps", tag="warm")
                for _ in range(8):
                    nc.tensor.matmul(wps, warm_sb[:, 0:128], warm_sb,
                                     start=True, stop=True)

            # ---- phase A+B: qkv projection pipelined with fmap stripe DMAs;
            # bias built mid-stream once QT is complete ----
            with tc.tile_pool(name="psA", bufs=2, space="PSUM") as psA:
                def qkv_group(dst, col0, t, eng):
                    ps = psA.tile([128, 1024], f32, name="qkv_ps", tag="qkv", bufs=2)
                    for c in range(4):
                        for h in range(2):
                            nc.tensor.matmul(
                                ps[:, h * 512:(h + 1) * 512],
                                W4[c][:, col0:col0 + 128],
                                F4[c][:, t * 1024 + h * 512: t * 1024 + (h + 1) * 512],
                                start=(c == 0), stop=(c == 3))
                    if eng == "act":
                        nc.scalar.copy(dst[:, t * 1024:(t + 1) * 1024], ps)
                    else:
                        nc.vector.tensor_copy(dst[:, t * 1024:(t + 1) * 1024], ps)

                def bias_matmuls():
                    # batched rel-logit matmuls: every shift-window at once
                    # into PSUM, evacuated whole to SBUF staging.
                    # bh_stage[p, q] = sum_d relh[d, p] * QT[d, q]
                    # bw_stage[p, (w*32+i)] = sum_d relw[d, p] * QT[d, iq(i,w)]
                    qt_w = QT.rearrange("d (i w) -> d w i", w=64)
                    for blk in range(4):
                        bh_ps = psA.tile([96, 512], f32, name="bh_ps", tag="bh", bufs=2)
                        nc.tensor.matmul(bh_ps, relh_sb,
                                         QT[:, blk * 512:(blk + 1) * 512],
                                         start=True, stop=True)
                        nc.vector.tensor_copy(
                            bh_stage[:, blk * 512:(blk + 1) * 512], bh_ps)
                        bw_ps = psA.tile([127, 512], f32, name="bw_ps", tag="bw", bufs=2)
                        nc.tensor.matmul(bw_ps, relw_sb,
                                         qt_w[:, blk * 16:(blk + 1) * 16, :],
                                         start=True, stop=True)
                        nc.scalar.copy(
                            bw_stage[:, blk * 512:(blk + 1) * 512], bw_ps)

                for t in range(4):
                    if t < 2:
                        qkv_group(QT, 0, t, "dve")
                    qkv_group(KT, 128, t, "act")
                    qkv_group(VTt, 256, t, "act")
                    if t == 1:
                        bias_matmuls()

                # shear: partition-offset SBUF->SBUF DMAs select the per-query
                # diagonal band of each staged window matrix, then one big DVE
                # copy converts to bf16 (and un-permutes bw's w-major order).
                for rr in range(32):
                    eng = nc.sync if rr % 2 == 0 else nc.scalar
                    eng.dma_start(
                        out=bh2[:, rr * 64:(rr + 1) * 64],
                        in_=bh_stage[31 - rr:95 - rr, rr * 64:(rr + 1) * 64])
                for w in range(64):
                    eng = nc.scalar if w % 2 == 0 else nc.sync
                    eng.dma_start(
                        out=bw2[:, w * 32:(w + 1) * 32],
                        in_=bw_stage[63 - w:127 - w, w * 32:(w + 1) * 32])
                nc.vector.tensor_copy(BT[0:64, :], bh2)
                nc.vector.tensor_copy(
                    BT[64:128, :].rearrange("p (i w) -> p i w", i=32, w=64),
                    bw2.rearrange("p (w i) -> p i w", w=64, i=32))

                # V transposes ride both HWDGE queues after the bulk input
                # DMAs and shear DMAs; chunk kc is needed ~1.4us * kc into
                # phase C, far behind this schedule.
                for s in range(32):
                    eng = nc.sync if s % 2 == 0 else nc.scalar
                    eng.dma_start_transpose(
                        Vn[:, s * 128:(s + 1) * 128],
                        VTt[:, s * 128:(s + 1) * 128])

            # ---- phase C: attention main loop ----
            with tc.tile_pool(name="psC", bufs=1, space="PSUM") as psC:
                for qb in range(2):
                    q0 = qb * QB
                    acc = work.tile([128, QB], fp16, name="acc", tag="acc", bufs=2)
                    acc2 = work.tile([128, QB], fp16, name="acc2", tag="acc2", bufs=2)
                    outT = psC.tile([128, QB], f32, name="outT", tag="out", bufs=2)
                    for kc in range(32):
                        sim = psC.tile([128, QB], f32, name="sim", tag="sim", bufs=2)
                        for h in range(2):
                            sl = slice(q0 + h * 512, q0 + (h + 1) * 512)
                            po = sim[:, h * 512:(h + 1) * 512]
                            nc.tensor.matmul(
                                po, KT[:, kc * 128:(kc + 1) * 128], QT[:, sl],
                                start=True, stop=False)
                            nc.tensor.matmul(
                                po, ind_sb[:, kc * 128:(kc + 1) * 128], BT[:, sl],
                                start=False, stop=True)
                        expT = work.tile([128, QB], fp16, name="expT", tag="exp", bufs=8)
                        nc.scalar.activation(expT, sim, EXPF, bias=bias4_sb[:, 0:1], scale=SCALE)
                        if kc == 31:
                            last_expT = expT  # reduced directly by the rowsum matmul
                        else:
                            a = acc if kc < 16 else acc2
                            if kc in (0, 16):
                                nc.vector.tensor_copy(a, expT)
                            else:
                                nc.vector.tensor_add(a, a, expT)
                        for h in range(2):
                            nc.tensor.matmul(
                                outT[:, h * 512:(h + 1) * 512],
                                Vn[:, kc * 128:(kc + 1) * 128],
                                expT[:, h * 512:(h + 1) * 512],
                                start=(kc == 0), stop=(kc == 31))

                    # normalize in pipelined 512-wide halves: rowsum
                    # (ones-matmul partition reduce) -> broadcast (K=1 outer
                    # product) -> approx reciprocal -> scale -> store
                    for hh in range(2):
                        sl = slice(hh * 512, (hh + 1) * 512)
                        rs_ps = psC.tile([1, 512], f32, name="rs_ps", tag="sim", bufs=2)
                        nc.tensor.matmul(rs_ps, onesh_sb[:, 0:1], acc[:, sl],
                                         start=True, stop=False)
                        nc.tensor.matmul(rs_ps, onesh_sb[:, 0:1], acc2[:, sl],
                                         start=False, stop=False)
                        nc.tensor.matmul(rs_ps, onesh_sb[:, 0:1], last_expT[:, sl],
                                         start=False, stop=True)
                        rs_row = work.tile([1, 512], fp16, name="rs_row", tag="rsrow", bufs=2)
                        nc.scalar.copy(rs_row, rs_ps)
                        bc_ps = psC.tile([128, 512], f32, name="bc_ps", tag="sim", bufs=2)
                        nc.tensor.matmul(bc_ps, onesh_sb[0:1, :], rs_row,
                                         start=True, stop=True)
                        rec_sb = work.tile([128, 512], f32, name="rec_sb", tag="bc", bufs=2)
                        nc.vector.reciprocal_approx_fast(out=rec_sb, in_=bc_ps)
                        out_sb = work.tile([128, 512], f32, name="out_sb", tag="osb", bufs=2)
                        nc.vector.tensor_mul(out_sb, outT[:, sl], rec_sb)
                        eng = nc.sync if hh == 0 else nc.scalar
                        eng.dma_start(out=out_p[:, q0 + hh * 512:q0 + (hh + 1) * 512],
                                      in_=out_sb)

    nc.finalize()
    return nc


def _prep_core_inputs(fmap, w_qkv, rel_height, rel_width, core):
    bf = ml_dtypes.bfloat16
    h, half = core // 2, core % 2
    q0 = half * NQ
    perm = (np.arange(L) + q0) % L
    fmap_flat = fmap.reshape(C, L)
    fmap_core = np.ascontiguousarray(fmap_flat[:, perm]).astype(bf)
    rows = np.r_[h * 128:(h + 1) * 128,
                 512 + h * 128:512 + (h + 1) * 128,
                 1024 + h * 128:1024 + (h + 1) * 128]
    wt = np.ascontiguousarray(w_qkv[rows].T).astype(bf)
    relhT = rel_height.T  # (128, 127)
    a = 32 * (1 - half)
    relh_slab = np.zeros((128, 96), np.float32)
    relh_slab[:, :95] = relhT[:, a:a + 95]
    relw = np.ascontiguousarray(rel_width.T).astype(bf)
    j = np.arange(L)
    ind = np.zeros((128, L), np.float32)
    ind[(j // 64 + 32 * half) % 64, j] = 1.0
    ind[64 + (j % 64), j] = 1.0
    fmap_blocks = np.ascontiguousarray(
        fmap_core.reshape(4, 128, 4, 1024).transpose(0, 2, 1, 3).reshape(16 * 128, 1024))
    ind_blocks = np.ascontiguousarray(
        ind.reshape(128, 4, 1024).transpose(1, 0, 2).reshape(4 * 128, 1024))

    return {
        "fmapc": fmap_blocks,
        "wt": wt,
        "relh": relh_slab.astype(bf),
        "relw": relw,
        "ind": ind_blocks.astype(bf),
        "onesh": np.ones((128, 128), np.float16),
        "bias4": np.full((128, 1), -4.0, np.float32),
    }


def _install_trace_hook():
    """Register the axon NTFF profiling hook (missing antenv.axon_hooks shim)
    and neuter the artifact upload so tracing works in this sandbox."""
    import sys
    import types
    import concourse.bass_utils as bu
    bu.upload_artifacts = lambda d: d
    try:
        from antenv import axon_hooks  # noqa: F401
        return
    except ImportError:
        pass
    import antenv
    mod = types.ModuleType("antenv.axon_hooks")
    mod._hook = None
    def set_axon_ntff_profile_hook(h):
        mod._hook = h
    def get_axon_ntff_profile_hook():
        return mod._hook
    mod.set_axon_ntff_profile_hook = set_axon_ntff_profile_hook
    mod.get_axon_ntff_profile_hook = get_axon_ntff_profile_hook
    sys.modules["antenv.axon_hooks"] = mod
    antenv.axon_hooks = mod
    try:
        from trn_agent_boot.trn_boot import _ntff_profile_via_ctypes
        h = _ntff_profile_via_ctypes("/opt/axon/libaxon_pjrt.so")
        if h is not None:
            mod._hook = h
    except Exception as e:
        print(f"trace hook install failed: {e}")


def kernel(fmap, w_qkv, rel_height, rel_width, _trace=False):
    global _GRAPH
    from concourse.bass_utils import run_bass_kernel_spmd

    fmap = np.asarray(fmap, dtype=np.float32)
    w_qkv = np.asarray(w_qkv, dtype=np.float32)
    rel_height = np.asarray(rel_height, dtype=np.float32)
    rel_width = np.asarray(rel_width, dtype=np.float32)

    if _GRAPH is None:
        _GRAPH = _build_graph()
    nc = _GRAPH

    in_maps = [_prep_core_inputs(fmap, w_qkv, rel_height, rel_width, c)
               for c in range(NCORES)]
    kw = {}
    if _trace:
        _install_trace_hook()
        import os
        os.makedirs("/tmp/ktrace", exist_ok=True)
        import tempfile
        kw = dict(tmpdir=tempfile.mkdtemp(dir="/tmp/ktrace"))
    res = None
    last_err = None
    for attempt in range(3):
        try:
            res = run_bass_kernel_spmd(nc, in_maps, core_ids=list(range(NCORES)),
                                       trace=_trace, **kw)
            break
        except Exception as e:  # transient PJRT/tunnel hiccups
            last_err = e
    if res is None:
        raise last_err
    out_full = np.zeros((C, L), np.float32)
    for c in range(NCORES):
        h, half = c // 2, c % 2
        out_full[h * 128:(h + 1) * 128, half * NQ:(half + 1) * NQ] = \
            np.asarray(res.results[c]["out"])
    if _trace:
        kernel._last_exec_time_ns = res.exec_time_ns
        kernel._last_profile = res.profile_json
    return out_full.reshape(1, C, H, W)


# revision 10
# speedup vs baseline: 1.0761x; 1.0761x over previous
"""Trainium2 Bass kernel for BotNet-style sparse attention (4 heads, 64x64 map,
dh=128, decomposed 2D relative position bias).

Sharding: 8 cores = 4 heads x 2 query-halves. Each core computes its head's
q/k/v from the full fmap, builds the rel-pos bias row tensors on chip, and runs
flash-style attention in "transposed sim" orientation (keys on partitions,
queries on free dim) so no attention-matrix transposes are needed:

  simT[k, q] = K^T.T @ Q^T  (+ bias via indicator-matmul accumulation)
  expT = exp(SCALE * simT - 4)           (ACT, PSUM->SBUF fp16)
  outT[d, q] = sum_k V[k, d] * expT[k,q] (PSUM accumulation over key chunks)
  rowsum via DVE accumulate + ones-matmul partition reduce
  out = outT * (1/rowsum) broadcast      (K=1 outer-product matmul broadcast)

The rel-pos bias decomposes per query q=(hq,wq), key k=(hk,wk) as
  bias = Rh[q, hk-hq+63] + Rw[q, wk-wq+63]
The row tensors BT are built from two batched matmuls producing every rel
window at once in PSUM, evacuated whole to SBUF, sheared into per-query
diagonal bands with partition-offset SBUF->SBUF DMAs, and converted to bf16
by single big DVE copies. The bias then folds into sim via one accumulating
matmul against a 0/1 indicator matrix per key chunk.

Per-core inputs are key-permuted (own query half first) so the SPMD graph is
identical across cores; all per-core differences live in the input data.
"""

import numpy as np
import ml_dtypes

C, H, W = 512, 64, 64
HEADS, DH = 4, 128
L = H * W           # 4096
NQ = L // 2         # 2048 queries per core
QB = 1024           # query block
SCALE = DH ** -0.5
NCORES = 8

_GRAPH = None


def _build_graph():
    from concourse import bacc
    import concourse.mybir as mybir
    import concourse.tile as tile
    from concourse.ap import AP as RawAP

    f32 = mybir.dt.float32
    bf16 = mybir.dt.bfloat16
    fp16 = mybir.dt.float16
    EXPF = mybir.ActivationFunctionType.Exp

    nc = bacc.Bacc(None)

    bh_d = nc.dram_tensor("bh_d", [96, NQ], f32, kind="Internal")
    bw_d = nc.dram_tensor("bw_d", [127, NQ], f32, kind="Internal")

    fmap_p = nc.declare_dram_parameter("fmapc", [16 * 128, 1024], bf16, isOutput=False)
    wt_p = nc.declare_dram_parameter("wt", [C, 384], bf16, isOutput=False)
    relh_p = nc.declare_dram_parameter("relh", [128, 96], bf16, isOutput=False)
    relw_p = nc.declare_dram_parameter("relw", [128, 127], bf16, isOutput=False)
    ind_p = nc.declare_dram_parameter("ind", [4 * 128, 1024], bf16, isOutput=False)
    onesh_p = nc.declare_dram_parameter("onesh", [128, 128], fp16, isOutput=False)
    bias4_p = nc.declare_dram_parameter("bias4", [128, 1], f32, isOutput=False)
    out_p = nc.declare_dram_parameter("out", [128, NQ], f32, isOutput=True)

    with tile.TileContext(nc) as tc:
        with tc.tile_pool(name="const", bufs=1) as cpool, \
             tc.tile_pool(name="big", bufs=1) as big, \
             tc.tile_pool(name="work", bufs=2) as work:

            # warm tile memset first in the gpsimd stream so PE warmup
            # matmuls can start right after the init barrier
            warm_sb = work.tile([128, 512], bf16, name="warm_sb", tag="warm")
            nc.gpsimd.memset(warm_sb, 0.0)

            # ---- constants to SBUF (small; on the slow SWDGE queue) ----
            relh_sb = cpool.tile([128, 96], bf16, name="relh_sb")
            relw_sb = cpool.tile([128, 127], bf16, name="relw_sb")
            ind_sb = cpool.tile([128, L], bf16, name="ind_sb")
            onesh_sb = cpool.tile([128, 128], fp16, name="onesh_sb")
            bias4_sb = cpool.tile([128, 1], f32, name="bias4_sb")

            # ---- weights first (small, unblock qkv matmuls), then fmap
            # t-major so each 1024-column stripe completes across all four
            # c-tiles early; spread across engine DMA queues for bandwidth ----
            F4 = [big.tile([128, L], bf16, name=f"F{c}") for c in range(4)]
            W4 = []
            w_engs = [nc.sync, nc.scalar, nc.sync, nc.scalar]
            for c in range(4):
                w = big.tile([128, 384], bf16, name=f"W{c}")
                w_engs[c].dma_start(out=w, in_=wt_p[c * 128:(c + 1) * 128, :])
                W4.append(w)
            dma_engs = [nc.sync, nc.scalar, nc.scalar, nc.sync]
            def fblk(c, t):
                b = c * 4 + t
                return fmap_p[b * 128:(b + 1) * 128, :]

            for h in range(2):
                for c in range(4):
                    dma_engs[c].dma_start(
                        out=F4[c][:, h * 512:(h + 1) * 512],
                        in_=fblk(c, 0)[:, h * 512:(h + 1) * 512])
            for t in range(1, 4):
                for c in range(4):
                    dma_engs[c].dma_start(
                        out=F4[c][:, t * 1024:(t + 1) * 1024], in_=fblk(c, t))
            for k in range(4):
                (nc.sync if k % 2 == 0 else nc.scalar).dma_start(
                    out=ind_sb[:, k * 1024:(k + 1) * 1024],
                    in_=ind_p[k * 128:(k + 1) * 128, :])
            nc.gpsimd.dma_start(out=relh_sb, in_=relh_p[:, :])
            nc.gpsimd.dma_start(out=relw_sb, in_=relw_p[:, :])
            nc.gpsimd.dma_start(out=onesh_sb, in_=onesh_p[:, :])
            nc.gpsimd.dma_start(out=bias4_sb, in_=bias4_p[:, :])

            QT = big.tile([128, NQ], bf16, name="QT")
            KT = big.tile([128, L], bf16, name="KT")
            VTt = big.tile([128, L], bf16, name="VTt")
            Vn = big.tile([128, L], bf16, name="Vn")
            BT = big.tile([128, NQ], bf16, name="BT")
            bh_stage = big.tile([96, NQ], f32, name="bh_stage")
            bw_stage = big.tile([127, NQ], f32, name="bw_stage")
            bh2 = big.tile([64, NQ], f32, name="bh2")
            bw2 = big.tile([64, NQ], f32, name="bw2")

            # ---- PE warmup: dummy matmuls on the memset tile fill the early
            # DMA wait and open the HAM clock-gate before real matmuls ----
            with tc.tile_pool(name="psW", bufs=1, space="PSUM") as psW:
                wps = psW.tile([128, 512], f32, name="warm_ps", tag="warm")
                for _ in range(8):
                    nc.tensor.matmul(wps, warm_sb[:, 0:128], warm_sb,
                                     start=True, stop=True)

            # ---- phase A+B: qkv projection pipelined with fmap stripe DMAs;
            # bias built mid-stream once QT is complete ----
            with tc.tile_pool(name="psA", bufs=2, space="PSUM") as psA:
                def qkv_group(dst, col0, t, eng):
                    ps = psA.tile([128, 1024], f32, name="qkv_ps", tag="qkv", bufs=2)
                    for c in range(4):
                        for h in range(2):
                            nc.tensor.matmul(
                                ps[:, h * 512:(h + 1) * 512],
                                W4[c][:, col0:col0 + 128],
                                F4[c][:, t * 1024 + h * 512: t * 1024 + (h + 1) * 512],
                                start=(c == 0), stop=(c == 3))
                    if eng == "act":
                        nc.scalar.copy(dst[:, t * 1024:(t + 1) * 1024], ps)
                    else:
                        nc.vector.tensor_copy(dst[:, t * 1024:(t + 1) * 1024], ps)

                def bias_matmuls():
                    # batched rel-logit matmuls: every shift-window at once
                    # into PSUM, evacuated whole to SBUF staging, staged out
                    # to DRAM scratch (window order reversed via host-side
                    # reversed rel tables so the shear strides are positive).
                    # bh_stage[m, q] = sum_d relh_rev[d, m] * QT[d, q]
                    # bw_stage[m, (w*32+i)] = sum_d relw_rev[d, m] * QT[d, q(i,w)]
                    qt_w = QT.rearrange("d (i w) -> d w i", w=64)
                    for blk in range(4):
                        bh_ps = psA.tile([96, 512], f32, name="bh_ps", tag="bh", bufs=2)
                        nc.tensor.matmul(bh_ps, relh_sb,
                                         QT[:, blk * 512:(blk + 1) * 512],
                                         start=True, stop=True)
                        nc.vector.tensor_copy(
                            bh_stage[:, blk * 512:(blk + 1) * 512], bh_ps)
                        nc.sync.dma_start(
                            out=bh_d[:, blk * 512:(blk + 1) * 512],
                            in_=bh_stage[:, blk * 512:(blk + 1) * 512])
                        bw_ps = psA.tile([127, 512], f32, name="bw_ps", tag="bw", bufs=2)
                        nc.tensor.matmul(bw_ps, relw_sb,
                                         qt_w[:, blk * 16:(blk + 1) * 16, :],
                                         start=True, stop=True)
                        nc.scalar.copy(
                            bw_stage[:, blk * 512:(blk + 1) * 512], bw_ps)
                        nc.scalar.dma_start(
                            out=bw_d[:, blk * 512:(blk + 1) * 512],
                            in_=bw_stage[:, blk * 512:(blk + 1) * 512])

                def v_transposes(s0, s1):
                    for s in range(s0, s1):
                        eng = nc.sync if s % 2 == 0 else nc.scalar
                        eng.dma_start_transpose(
                            Vn[:, s * 128:(s + 1) * 128],
                            VTt[:, s * 128:(s + 1) * 128])

                for t in range(4):
                    if t < 2:
                        qkv_group(QT, 0, t, "dve")
                    qkv_group(KT, 128, t, "act")
                    qkv_group(VTt, 256, t, "act")
                    if t == 1:
                        bias_matmuls()
                        # single strided shear reads: with reversed window
                        # order, src flat index is (1+p+rr)*2048 + rr*64 + w
                        # for bh and (c+w)*2048 + w*32 + i for bw — all
                        # positive strides, contiguous innermost runs.
                        nc.sync.dma_start(
                            out=bh2.rearrange("p (r w) -> p r w", r=32, w=64),
                            in_=RawAP(bh_d, NQ, [[NQ, 64], [NQ + 64, 32], [1, 64]]))
                        nc.scalar.dma_start(
                            out=bw2.rearrange("p (w i) -> p w i", w=64, i=32),
                            in_=RawAP(bw_d, 0, [[NQ, 64], [NQ + 32, 64], [1, 32]]))
                        nc.vector.tensor_copy(BT[0:64, :], bh2)
                        nc.vector.tensor_copy(
                            BT[64:128, :].rearrange("p (i w) -> p i w", i=32, w=64),
                            bw2.rearrange("p (w i) -> p i w", w=64, i=32))
                    if t >= 1:
                        # V transposes ride both HWDGE queues behind each
                        # stripe's qkv; chunk kc is needed ~1.5us * kc into
                        # phase C, far behind this schedule.
                        v_transposes((t - 1) * 8, t * 8)
                v_transposes(24, 32)

            # ---- phase C: attention main loop ----
            with tc.tile_pool(name="psC", bufs=1, space="PSUM") as psC:
                for qb in range(2):
                    q0 = qb * QB
                    acc = work.tile([128, QB], fp16, name="acc", tag="acc", bufs=2)
                    acc2 = work.tile([128, QB], fp16, name="acc2", tag="acc2", bufs=2)
                    outT = psC.tile([128, QB], f32, name="outT", tag="out", bufs=2)
                    for kc in range(32):
                        sim = psC.tile([128, QB], f32, name="sim", tag="sim", bufs=2)
                        for h in range(2):
                            sl = slice(q0 + h * 512, q0 + (h + 1) * 512)
                            po = sim[:, h * 512:(h + 1) * 512]
                            nc.tensor.matmul(
                                po, KT[:, kc * 128:(kc + 1) * 128], QT[:, sl],
                                start=True, stop=False)
                            nc.tensor.matmul(
                                po, ind_sb[:, kc * 128:(kc + 1) * 128], BT[:, sl],
                                start=False, stop=True)
                        expT = work.tile([128, QB], fp16, name="expT", tag="exp", bufs=8)
                        nc.scalar.activation(expT, sim, EXPF, bias=bias4_sb[:, 0:1], scale=SCALE)
                        if kc == 31:
                            last_expT = expT  # reduced directly by the rowsum matmul
                        else:
                            a = acc if kc < 16 else acc2
                            if kc in (0, 16):
                                nc.vector.tensor_copy(a, expT)
                            else:
                                nc.vector.tensor_add(a, a, expT)
                        for h in range(2):
                            nc.tensor.matmul(
                                outT[:, h * 512:(h + 1) * 512],
                                Vn[:, kc * 128:(kc + 1) * 128],
                                expT[:, h * 512:(h + 1) * 512],
                                start=(kc == 0), stop=(kc == 31))

                    # normalize in pipelined 512-wide halves: rowsum
                    # (ones-matmul partition reduce) -> broadcast (K=1 outer
                    # product) -> approx reciprocal -> scale -> store
                    for hh in range(2):
                        sl = slice(hh * 512, (hh + 1) * 512)
                        rs_ps = psC.tile([1, 512], f32, name="rs_ps", tag="sim", bufs=2)
                        nc.tensor.matmul(rs_ps, onesh_sb[:, 0:1], acc[:, sl],
                                         start=True, stop=False)
                        nc.tensor.matmul(rs_ps, onesh_sb[:, 0:1], acc2[:, sl],
                                         start=False, stop=False)
                        nc.tensor.matmul(rs_ps, onesh_sb[:, 0:1], last_expT[:, sl],
                                         start=False, stop=True)
                        rs_row = work.tile([1, 512], fp16, name="rs_row", tag="rsrow", bufs=2)
                        nc.scalar.copy(rs_row, rs_ps)
                        bc_ps = psC.tile([128, 512], f32, name="bc_ps", tag="sim", bufs=2)
                        nc.tensor.matmul(bc_ps, onesh_sb[0:1, :], rs_row,
                                         start=True, stop=True)
                        rec_sb = work.tile([128, 512], f32, name="rec_sb", tag="bc", bufs=2)
                        nc.vector.reciprocal_approx_fast(out=rec_sb, in_=bc_ps)
                        out_sb = work.tile([128, 512], f32, name="out_sb", tag="osb", bufs=2)
                        nc.vector.tensor_mul(out_sb, outT[:, sl], rec_sb)
                        eng = nc.sync if hh == 0 else nc.scalar
                        eng.dma_start(out=out_p[:, q0 + hh * 512:q0 + (hh + 1) * 512],
                                      in_=out_sb)

    nc.finalize()
    return nc


def _prep_core_inputs(fmap, w_qkv, rel_height, rel_width, core):
    bf = ml_dtypes.bfloat16
    h, half = core // 2, core % 2
    q0 = half * NQ
    perm = (np.arange(L) + q0) % L
    fmap_flat = fmap.reshape(C, L)
    fmap_core = np.ascontiguousarray(fmap_flat[:, perm]).astype(bf)
    rows = np.r_[h * 128:(h + 1) * 128,
                 512 + h * 128:512 + (h + 1) * 128,
                 1024 + h * 128:1024 + (h + 1) * 128]
    wt = np.ascontiguousarray(w_qkv[rows].T).astype(bf)
    relhT = rel_height.T  # (128, 127)
    a = 32 * (1 - half)
    relh_slab = np.zeros((128, 96), np.float32)
    relh_slab[:, :95] = relhT[:, a:a + 95]
    relh_slab = relh_slab[:, ::-1]  # reversed window order (shear strides > 0)
    relw = np.ascontiguousarray(rel_width.T[:, ::-1]).astype(bf)
    j = np.arange(L)
    ind = np.zeros((128, L), np.float32)
    ind[63 - (j // 64 + 32 * half) % 64, j] = 1.0
    ind[64 + 63 - (j % 64), j] = 1.0
    fmap_blocks = np.ascontiguousarray(
        fmap_core.reshape(4, 128, 4, 1024).transpose(0, 2, 1, 3).reshape(16 * 128, 1024))
    ind_blocks = np.ascontiguousarray(
        ind.reshape(128, 4, 1024).transpose(1, 0, 2).reshape(4 * 128, 1024))

    return {
        "fmapc": fmap_blocks,
        "wt": wt,
        "relh": relh_slab.astype(bf),
        "relw": relw,
        "ind": ind_blocks.astype(bf),
        "onesh": np.ones((128, 128), np.float16),
        "bias4": np.full((128, 1), -4.0, np.float32),
    }


def _install_trace_hook():
    """Register the axon NTFF profiling hook (missing antenv.axon_hooks shim)
    and neuter the artifact upload so tracing works in this sandbox."""
    import sys
    import types
    import concourse.bass_utils as bu
    bu.upload_artifacts = lambda d: d
    try:
        from antenv import axon_hooks  # noqa: F401
        return
    except ImportError:
        pass
    import antenv
    mod = types.ModuleType("antenv.axon_hooks")
    mod._hook = None
    def set_axon_ntff_profile_hook(h):
        mod._hook = h
    def get_axon_ntff_profile_hook():
        return mod._hook
    mod.set_axon_ntff_profile_hook = set_axon_ntff_profile_hook
    mod.get_axon_ntff_profile_hook = get_axon_ntff_profile_hook
    sys.modules["antenv.axon_hooks"] = mod
    antenv.axon_hooks = mod
    try:
        from trn_agent_boot.trn_boot import _ntff_profile_via_ctypes
        h = _ntff_profile_via_ctypes("/opt/axon/libaxon_pjrt.so")
        if h is not None:
            mod._hook = h
    except Exception as e:
        print(f"trace hook install failed: {e}")


def kernel(fmap, w_qkv, rel_height, rel_width, _trace=False):
    global _GRAPH
    from concourse.bass_utils import run_bass_kernel_spmd

    fmap = np.asarray(fmap, dtype=np.float32)
    w_qkv = np.asarray(w_qkv, dtype=np.float32)
    rel_height = np.asarray(rel_height, dtype=np.float32)
    rel_width = np.asarray(rel_width, dtype=np.float32)

    if _GRAPH is None:
        _GRAPH = _build_graph()
    nc = _GRAPH

    in_maps = [_prep_core_inputs(fmap, w_qkv, rel_height, rel_width, c)
               for c in range(NCORES)]
    kw = {}
    if _trace:
        _install_trace_hook()
        import os
        os.makedirs("/tmp/ktrace", exist_ok=True)
        import tempfile
        kw = dict(tmpdir=tempfile.mkdtemp(dir="/tmp/ktrace"))
    res = None
    last_err = None
    for attempt in range(3):
        try:
            res = run_bass_kernel_spmd(nc, in_maps, core_ids=list(range(NCORES)),
                                       trace=_trace, **kw)
            break
        except Exception as e:  # transient PJRT/tunnel hiccups
            last_err = e
    if res is None:
        raise last_err
    out_full = np.zeros((C, L), np.float32)
    for c in range(NCORES):
        h, half = c // 2, c % 2
        out_full[h * 128:(h + 1) * 128, half * NQ:(half + 1) * NQ] = \
            np.asarray(res.results[c]["out"])
    if _trace:
        kernel._last_exec_time_ns = res.exec_time_ns
        kernel._last_profile = res.profile_json
    return out_full.reshape(1, C, H, W)


# revision 11
# speedup vs baseline: 1.0821x; 1.0056x over previous
"""Trainium2 Bass kernel for BotNet-style sparse attention (4 heads, 64x64 map,
dh=128, decomposed 2D relative position bias).

Sharding: 8 cores = 4 heads x 2 query-halves. Each core computes its head's
q/k/v from the full fmap, builds the rel-pos bias row tensors on chip, and runs
flash-style attention in "transposed sim" orientation (keys on partitions,
queries on free dim) so no attention-matrix transposes are needed:

  simT[k, q] = K^T.T @ Q^T  (+ bias via indicator-matmul accumulation)
  expT = exp(SCALE * simT - 4)           (ACT, PSUM->SBUF fp16)
  outT[d, q] = sum_k V[k, d] * expT[k,q] (PSUM accumulation over key chunks)
  rowsum via DVE accumulate + ones-matmul partition reduce
  out = outT * (1/rowsum) broadcast      (K=1 outer-product matmul broadcast)

The rel-pos bias decomposes per query q=(hq,wq), key k=(hk,wk) as
  bias = Rh[q, hk-hq+63] + Rw[q, wk-wq+63]
The row tensors BT are built from two batched matmuls producing every rel
window at once in PSUM, evacuated whole to SBUF, sheared into per-query
diagonal bands with partition-offset SBUF->SBUF DMAs, and converted to bf16
by single big DVE copies. The bias then folds into sim via one accumulating
matmul against a 0/1 indicator matrix per key chunk.

Per-core inputs are key-permuted (own query half first) so the SPMD graph is
identical across cores; all per-core differences live in the input data.
"""

import numpy as np
import ml_dtypes

C, H, W = 512, 64, 64
HEADS, DH = 4, 128
L = H * W           # 4096
NQ = L // 2         # 2048 queries per core
QB = 1024           # query block
SCALE = DH ** -0.5
NCORES = 8

_GRAPH = None


def _build_graph():
    from concourse import bacc
    import concourse.mybir as mybir
    import concourse.tile as tile
    from concourse.ap import AP as RawAP

    f32 = mybir.dt.float32
    bf16 = mybir.dt.bfloat16
    fp16 = mybir.dt.float16
    EXPF = mybir.ActivationFunctionType.Exp

    nc = bacc.Bacc(None)

    bh_d = nc.dram_tensor("bh_d", [96, NQ], f32, kind="Internal")
    bw_d = nc.dram_tensor("bw_d", [127, NQ], f32, kind="Internal")

    fmap_p = nc.declare_dram_parameter("fmapc", [16 * 128, 1024], bf16, isOutput=False)
    wt_p = nc.declare_dram_parameter("wt", [C, 384], bf16, isOutput=False)
    relh_p = nc.declare_dram_parameter("relh", [128, 96], bf16, isOutput=False)
    relw_p = nc.declare_dram_parameter("relw", [128, 127], bf16, isOutput=False)
    ind_p = nc.declare_dram_parameter("ind", [4 * 128, 1024], bf16, isOutput=False)
    onesh_p = nc.declare_dram_parameter("onesh", [128, 128], fp16, isOutput=False)
    bias4_p = nc.declare_dram_parameter("bias4", [128, 1], f32, isOutput=False)
    out_p = nc.declare_dram_parameter("out", [128, NQ], f32, isOutput=True)

    with tile.TileContext(nc) as tc:
        with tc.tile_pool(name="const", bufs=1) as cpool, \
             tc.tile_pool(name="big", bufs=1) as big, \
             tc.tile_pool(name="work", bufs=2) as work:

            # warm tile memset first in the gpsimd stream so PE warmup
            # matmuls can start right after the init barrier
            warm_sb = work.tile([128, 512], bf16, name="warm_sb", tag="warm")
            nc.gpsimd.memset(warm_sb, 0.0)

            # ---- constants to SBUF (small; on the slow SWDGE queue) ----
            relh_sb = cpool.tile([128, 96], bf16, name="relh_sb")
            relw_sb = cpool.tile([128, 127], bf16, name="relw_sb")
            ind_sb = cpool.tile([128, L], bf16, name="ind_sb")
            onesh_sb = cpool.tile([128, 128], fp16, name="onesh_sb")
            bias4_sb = cpool.tile([128, 1], f32, name="bias4_sb")

            # ---- weights first (small, unblock qkv matmuls), then fmap
            # t-major so each 1024-column stripe completes across all four
            # c-tiles early; spread across engine DMA queues for bandwidth ----
            F4 = [big.tile([128, L], bf16, name=f"F{c}") for c in range(4)]
            W4 = []
            w_engs = [nc.sync, nc.scalar, nc.sync, nc.scalar]
            for c in range(4):
                w = big.tile([128, 384], bf16, name=f"W{c}")
                w_engs[c].dma_start(out=w, in_=wt_p[c * 128:(c + 1) * 128, :])
                W4.append(w)
            dma_engs = [nc.sync, nc.scalar, nc.scalar, nc.sync]
            def fblk(c, t):
                b = c * 4 + t
                return fmap_p[b * 128:(b + 1) * 128, :]

            for h in range(2):
                for c in range(4):
                    dma_engs[c].dma_start(
                        out=F4[c][:, h * 512:(h + 1) * 512],
                        in_=fblk(c, 0)[:, h * 512:(h + 1) * 512])
            for t in range(1, 4):
                for c in range(4):
                    dma_engs[c].dma_start(
                        out=F4[c][:, t * 1024:(t + 1) * 1024], in_=fblk(c, t))
            for k in range(4):
                (nc.sync if k % 2 == 0 else nc.scalar).dma_start(
                    out=ind_sb[:, k * 1024:(k + 1) * 1024],
                    in_=ind_p[k * 128:(k + 1) * 128, :])
            nc.gpsimd.dma_start(out=relh_sb, in_=relh_p[:, :])
            nc.gpsimd.dma_start(out=relw_sb, in_=relw_p[:, :])
            nc.gpsimd.dma_start(out=onesh_sb, in_=onesh_p[:, :])
            nc.gpsimd.dma_start(out=bias4_sb, in_=bias4_p[:, :])

            QT = big.tile([128, NQ], bf16, name="QT")
            KT = big.tile([128, L], bf16, name="KT")
            VTt = big.tile([128, L], bf16, name="VTt")
            Vn = big.tile([128, L], bf16, name="Vn")
            BT = big.tile([128, NQ], bf16, name="BT")
            bh_stage = big.tile([96, NQ], f32, name="bh_stage")
            bw_stage = big.tile([127, NQ], f32, name="bw_stage")
            bh2 = big.tile([64, NQ], f32, name="bh2")
            bw2 = big.tile([64, NQ], f32, name="bw2")

            # ---- PE warmup: dummy matmuls on the memset tile fill the early
            # DMA wait and open the HAM clock-gate before real matmuls ----
            with tc.tile_pool(name="psW", bufs=1, space="PSUM") as psW:
                wps = psW.tile([128, 512], f32, name="warm_ps", tag="warm")
                for _ in range(8):
                    nc.tensor.matmul(wps, warm_sb[:, 0:128], warm_sb,
                                     start=True, stop=True)

            # ---- phase A+B: qkv projection pipelined with fmap stripe DMAs;
            # bias built mid-stream once QT is complete ----
            with tc.tile_pool(name="psA", bufs=2, space="PSUM") as psA:
                def qkv_group(dst, col0, t, eng):
                    ps = psA.tile([128, 1024], f32, name="qkv_ps", tag="qkv", bufs=2)
                    for c in range(4):
                        for h in range(2):
                            nc.tensor.matmul(
                                ps[:, h * 512:(h + 1) * 512],
                                W4[c][:, col0:col0 + 128],
                                F4[c][:, t * 1024 + h * 512: t * 1024 + (h + 1) * 512],
                                start=(c == 0), stop=(c == 3))
                    if eng == "act":
                        nc.scalar.copy(dst[:, t * 1024:(t + 1) * 1024], ps)
                    else:
                        nc.vector.tensor_copy(dst[:, t * 1024:(t + 1) * 1024], ps)

                def bias_matmuls():
                    # batched rel-logit matmuls: every shift-window at once
                    # into PSUM, evacuated whole to SBUF staging, staged out
                    # to DRAM scratch (window order reversed via host-side
                    # reversed rel tables so the shear strides are positive).
                    # bh_stage[m, q] = sum_d relh_rev[d, m] * QT[d, q]
                    # bw_stage[m, (w*32+i)] = sum_d relw_rev[d, m] * QT[d, q(i,w)]
                    qt_w = QT.rearrange("d (i w) -> d w i", w=64)
                    for blk in range(4):
                        bh_ps = psA.tile([96, 512], f32, name="bh_ps", tag="bh", bufs=2)
                        nc.tensor.matmul(bh_ps, relh_sb,
                                         QT[:, blk * 512:(blk + 1) * 512],
                                         start=True, stop=True)
                        nc.vector.tensor_copy(
                            bh_stage[:, blk * 512:(blk + 1) * 512], bh_ps)
                        nc.sync.dma_start(
                            out=bh_d[:, blk * 512:(blk + 1) * 512],
                            in_=bh_stage[:, blk * 512:(blk + 1) * 512])
                        bw_ps = psA.tile([127, 512], f32, name="bw_ps", tag="bw", bufs=2)
                        nc.tensor.matmul(bw_ps, relw_sb,
                                         qt_w[:, blk * 16:(blk + 1) * 16, :],
                                         start=True, stop=True)
                        nc.scalar.copy(
                            bw_stage[:, blk * 512:(blk + 1) * 512], bw_ps)
                        nc.scalar.dma_start(
                            out=bw_d[:, blk * 512:(blk + 1) * 512],
                            in_=bw_stage[:, blk * 512:(blk + 1) * 512])

                def v_transposes(s0, s1):
                    for s in range(s0, s1):
                        eng = nc.sync if s % 2 == 0 else nc.scalar
                        eng.dma_start_transpose(
                            Vn[:, s * 128:(s + 1) * 128],
                            VTt[:, s * 128:(s + 1) * 128])

                # both Q stripes first so the bias pipeline (matmuls -> PSUM
                # evac -> DRAM stage-out -> slow strided shear read-back)
                # launches early and overlaps the remaining K/V projection.
                qkv_group(QT, 0, 0, "dve")
                qkv_group(KT, 128, 0, "act")
                qkv_group(QT, 0, 1, "dve")
                qkv_group(VTt, 256, 0, "act")
                bias_matmuls()
                # single strided shear reads: with reversed window order,
                # src flat index is (1+p+rr)*2048 + rr*64 + w for bh and
                # (c+w)*2048 + w*32 + i for bw — all positive strides,
                # contiguous innermost runs, one DMA per queue.
                nc.sync.dma_start(
                    out=bh2.rearrange("p (r w) -> p r w", r=32, w=64),
                    in_=RawAP(bh_d, NQ, [[NQ, 64], [NQ + 64, 32], [1, 64]]))
                nc.scalar.dma_start(
                    out=bw2.rearrange("p (w i) -> p w i", w=64, i=32),
                    in_=RawAP(bw_d, 0, [[NQ, 64], [NQ + 32, 64], [1, 32]]))
                nc.vector.tensor_copy(BT[0:64, :], bh2)
                nc.vector.tensor_copy(
                    BT[64:128, :].rearrange("p (i w) -> p i w", i=32, w=64),
                    bw2.rearrange("p (w i) -> p i w", w=64, i=32))
                for t in range(1, 4):
                    qkv_group(KT, 128, t, "act")
                    qkv_group(VTt, 256, t, "act")
                    # V transposes ride both HWDGE queues behind the shear
                    # reads; chunk kc is needed ~1.5us * kc into phase C,
                    # far behind this schedule.
                    v_transposes((t - 1) * 8, t * 8)
                v_transposes(24, 32)

            # ---- phase C: attention main loop ----
            with tc.tile_pool(name="psC", bufs=1, space="PSUM") as psC:
                for qb in range(2):
                    q0 = qb * QB
                    acc = work.tile([128, QB], fp16, name="acc", tag="acc", bufs=2)
                    acc2 = work.tile([128, QB], fp16, name="acc2", tag="acc2", bufs=2)
                    outT = psC.tile([128, QB], f32, name="outT", tag="out", bufs=2)
                    for kc in range(32):
                        sim = psC.tile([128, QB], f32, name="sim", tag="sim", bufs=2)
                        for h in range(2):
                            sl = slice(q0 + h * 512, q0 + (h + 1) * 512)
                            po = sim[:, h * 512:(h + 1) * 512]
                            nc.tensor.matmul(
                                po, KT[:, kc * 128:(kc + 1) * 128], QT[:, sl],
                                start=True, stop=False)
                            nc.tensor.matmul(
                                po, ind_sb[:, kc * 128:(kc + 1) * 128], BT[:, sl],
                                start=False, stop=True)
                        expT = work.tile([128, QB], fp16, name="expT", tag="exp", bufs=8)
                        nc.scalar.activation(expT, sim, EXPF, bias=bias4_sb[:, 0:1], scale=SCALE)
                        if kc == 31:
                            last_expT = expT  # reduced directly by the rowsum matmul
                        else:
                            a = acc if kc < 16 else acc2
                            if kc in (0, 16):
                                nc.vector.tensor_copy(a, expT)
                            else:
                                nc.vector.tensor_add(a, a, expT)
                        for h in range(2):
                            nc.tensor.matmul(
                                outT[:, h * 512:(h + 1) * 512],
                                Vn[:, kc * 128:(kc + 1) * 128],
                                expT[:, h * 512:(h + 1) * 512],
                                start=(kc == 0), stop=(kc == 31))

                    # normalize in pipelined 512-wide halves: rowsum
                    # (ones-matmul partition reduce) -> broadcast (K=1 outer
                    # product) -> approx reciprocal -> scale -> store
                    for hh in range(2):
                        sl = slice(hh * 512, (hh + 1) * 512)
                        rs_ps = psC.tile([1, 512], f32, name="rs_ps", tag="sim", bufs=2)
                        nc.tensor.matmul(rs_ps, onesh_sb[:, 0:1], acc[:, sl],
                                         start=True, stop=False)
                        nc.tensor.matmul(rs_ps, onesh_sb[:, 0:1], acc2[:, sl],
                                         start=False, stop=False)
                        nc.tensor.matmul(rs_ps, onesh_sb[:, 0:1], last_expT[:, sl],
                                         start=False, stop=True)
                        rs_row = work.tile([1, 512], fp16, name="rs_row", tag="rsrow", bufs=2)
                        nc.scalar.copy(rs_row, rs_ps)
                        bc_ps = psC.tile([128, 512], f32, name="bc_ps", tag="sim", bufs=2)
                        nc.tensor.matmul(bc_ps, onesh_sb[0:1, :], rs_row,
                                         start=True, stop=True)
                        rec_sb = work.tile([128, 512], f32, name="rec_sb", tag="bc", bufs=2)
                        nc.vector.reciprocal_approx_fast(out=rec_sb, in_=bc_ps)
                        out_sb = work.tile([128, 512], f32, name="out_sb", tag="osb", bufs=2)
                        nc.vector.tensor_mul(out_sb, outT[:, sl], rec_sb)
                        eng = nc.sync if hh == 0 else nc.scalar
                        eng.dma_start(out=out_p[:, q0 + hh * 512:q0 + (hh + 1) * 512],
                                      in_=out_sb)

    nc.finalize()
    return nc


def _prep_core_inputs(fmap, w_qkv, rel_height, rel_width, core):
    bf = ml_dtypes.bfloat16
    h, half = core // 2, core % 2
    q0 = half * NQ
    perm = (np.arange(L) + q0) % L
    fmap_flat = fmap.reshape(C, L)
    fmap_core = np.ascontiguousarray(fmap_flat[:, perm]).astype(bf)
    rows = np.r_[h * 128:(h + 1) * 128,
                 512 + h * 128:512 + (h + 1) * 128,
                 1024 + h * 128:1024 + (h + 1) * 128]
    wt = np.ascontiguousarray(w_qkv[rows].T).astype(bf)
    relhT = rel_height.T  # (128, 127)
    a = 32 * (1 - half)
    relh_slab = np.zeros((128, 96), np.float32)
    relh_slab[:, :95] = relhT[:, a:a + 95]
    relh_slab = relh_slab[:, ::-1]  # reversed window order (shear strides > 0)
    relw = np.ascontiguousarray(rel_width.T[:, ::-1]).astype(bf)
    j = np.arange(L)
    ind = np.zeros((128, L), np.float32)
    ind[63 - (j // 64 + 32 * half) % 64, j] = 1.0
    ind[64 + 63 - (j % 64), j] = 1.0
    fmap_blocks = np.ascontiguousarray(
        fmap_core.reshape(4, 128, 4, 1024).transpose(0, 2, 1, 3).reshape(16 * 128, 1024))
    ind_blocks = np.ascontiguousarray(
        ind.reshape(128, 4, 1024).transpose(1, 0, 2).reshape(4 * 128, 1024))

    return {
        "fmapc": fmap_blocks,
        "wt": wt,
        "relh": relh_slab.astype(bf),
        "relw": relw,
        "ind": ind_blocks.astype(bf),
        "onesh": np.ones((128, 128), np.float16),
        "bias4": np.full((128, 1), -4.0, np.float32),
    }


def _install_trace_hook():
    """Register the axon NTFF profiling hook (missing antenv.axon_hooks shim)
    and neuter the artifact upload so tracing works in this sandbox."""
    import sys
    import types
    import concourse.bass_utils as bu
    bu.upload_artifacts = lambda d: d
    try:
        from antenv import axon_hooks  # noqa: F401
        return
    except ImportError:
        pass
    import antenv
    mod = types.ModuleType("antenv.axon_hooks")
    mod._hook = None
    def set_axon_ntff_profile_hook(h):
        mod._hook = h
    def get_axon_ntff_profile_hook():
        return mod._hook
    mod.set_axon_ntff_profile_hook = set_axon_ntff_profile_hook
    mod.get_axon_ntff_profile_hook = get_axon_ntff_profile_hook
    sys.modules["antenv.axon_hooks"] = mod
    antenv.axon_hooks = mod
    try:
        from trn_agent_boot.trn_boot import _ntff_profile_via_ctypes
        h = _ntff_profile_via_ctypes("/opt/axon/libaxon_pjrt.so")
        if h is not None:
            mod._hook = h
    except Exception as e:
        print(f"trace hook install failed: {e}")


def kernel(fmap, w_qkv, rel_height, rel_width, _trace=False):
    global _GRAPH
    from concourse.bass_utils import run_bass_kernel_spmd

    fmap = np.asarray(fmap, dtype=np.float32)
    w_qkv = np.asarray(w_qkv, dtype=np.float32)
    rel_height = np.asarray(rel_height, dtype=np.float32)
    rel_width = np.asarray(rel_width, dtype=np.float32)

    if _GRAPH is None:
        _GRAPH = _build_graph()
    nc = _GRAPH

    in_maps = [_prep_core_inputs(fmap, w_qkv, rel_height, rel_width, c)
               for c in range(NCORES)]
    kw = {}
    if _trace:
        _install_trace_hook()
        import os
        os.makedirs("/tmp/ktrace", exist_ok=True)
        import tempfile
        kw = dict(tmpdir=tempfile.mkdtemp(dir="/tmp/ktrace"))
    res = None
    last_err = None
    for attempt in range(3):
        try:
            res = run_bass_kernel_spmd(nc, in_maps, core_ids=list(range(NCORES)),
                                       trace=_trace, **kw)
            break
        except Exception as e:  # transient PJRT/tunnel hiccups
            last_err = e
    if res is None:
        raise last_err
    out_full = np.zeros((C, L), np.float32)
    for c in range(NCORES):
        h, half = c // 2, c % 2
        out_full[h * 128:(h + 1) * 128, half * NQ:(half + 1) * NQ] = \
            np.asarray(res.results[c]["out"])
    if _trace:
        kernel._last_exec_time_ns = res.exec_time_ns
        kernel._last_profile = res.profile_json
    return out_full.reshape(1, C, H, W)


# revision 18
# speedup vs baseline: 1.1493x; 1.0621x over previous
"""Trainium2 Bass kernel for BotNet-style sparse attention (4 heads, 64x64 map,
dh=128, decomposed 2D relative position bias).

Sharding: 8 cores = 4 heads x 2 query-halves. Each core computes its head's
q/k/v from the full fmap, builds the rel-pos bias row tensors on chip, and runs
flash-style attention in "transposed sim" orientation (keys on partitions,
queries on free dim) so no attention-matrix transposes are needed:

  simT[k, q] = K^T.T @ Q^T  (+ bias via indicator-matmul accumulation)
  expT = exp(SCALE * simT - 4)           (ACT, PSUM->SBUF fp16)
  outT[d, q] = sum_k V[k, d] * expT[k,q] (PSUM accumulation over key chunks)
  rowsum via DVE accumulate + ones-matmul partition reduce
  out = outT * (1/rowsum) broadcast      (K=1 outer-product matmul broadcast)

The rel-pos bias decomposes per query q=(hq,wq), key k=(hk,wk) as
  bias = Rh[q, hk-hq+63] + Rw[q, wk-wq+63]
The row tensors BT are built from two batched matmuls producing every rel
window at once in PSUM, evacuated whole to SBUF, sheared into per-query
diagonal bands with partition-offset SBUF->SBUF DMAs, and converted to bf16
by single big DVE copies. The bias then folds into sim via one accumulating
matmul against a 0/1 indicator matrix per key chunk.

Per-core inputs are key-permuted (own query half first) so the SPMD graph is
identical across cores; all per-core differences live in the input data.
"""

import numpy as np
import ml_dtypes

C, H, W = 512, 64, 64
HEADS, DH = 4, 128
L = H * W           # 4096
NQ = L // 2         # 2048 queries per core
QB = 1024           # query block
SCALE = DH ** -0.5
NCORES = 8

_GRAPH = None


def _build_graph():
    from concourse import bacc
    import concourse.mybir as mybir
    import concourse.tile as tile
    from concourse.ap import AP as RawAP

    f32 = mybir.dt.float32
    bf16 = mybir.dt.bfloat16
    fp16 = mybir.dt.float16
    EXPF = mybir.ActivationFunctionType.Exp

    nc = bacc.Bacc(None)

    bh_d = nc.dram_tensor("bh_d", [96, NQ], f32, kind="Internal")
    bw_d = nc.dram_tensor("bw_d", [127, NQ], f32, kind="Internal")

    fmap_p = nc.declare_dram_parameter("fmapc", [16 * 128, 1024], bf16, isOutput=False)
    wt_p = nc.declare_dram_parameter("wt", [C, 384], bf16, isOutput=False)
    relh_p = nc.declare_dram_parameter("relh", [128, 96], bf16, isOutput=False)
    relw_p = nc.declare_dram_parameter("relw", [128, 127], bf16, isOutput=False)
    ind_p = nc.declare_dram_parameter("ind", [4 * 128, 1024], bf16, isOutput=False)
    onesh_p = nc.declare_dram_parameter("onesh", [128, 128], fp16, isOutput=False)
    ident_p = nc.declare_dram_parameter("ident", [128, 128], bf16, isOutput=False)
    bias4_p = nc.declare_dram_parameter("bias4", [128, 1], f32, isOutput=False)
    out_p = nc.declare_dram_parameter("out", [128, NQ], f32, isOutput=True)

    with tile.TileContext(nc) as tc:
        with tc.tile_pool(name="const", bufs=1) as cpool, \
             tc.tile_pool(name="big", bufs=1) as big, \
             tc.tile_pool(name="work", bufs=2) as work:

            # warm tile memset first in the gpsimd stream so PE warmup
            # matmuls can start right after the init barrier
            warm_sb = work.tile([128, 512], bf16, name="warm_sb", tag="warm")
            nc.gpsimd.memset(warm_sb, 0.0)

            # ---- constants to SBUF (small; on the slow SWDGE queue) ----
            relh_sb = cpool.tile([128, 96], bf16, name="relh_sb")
            relw_sb = cpool.tile([128, 127], bf16, name="relw_sb")
            ind_sb = cpool.tile([128, L], bf16, name="ind_sb")
            onesh_sb = cpool.tile([128, 128], fp16, name="onesh_sb")
            ident_sb = cpool.tile([128, 128], bf16, name="ident_sb")
            bias4_sb = cpool.tile([128, 1], f32, name="bias4_sb")

            # ---- weights first (small, unblock qkv matmuls), then fmap
            # t-major so each 1024-column stripe completes across all four
            # c-tiles early; spread across engine DMA queues for bandwidth ----
            F4 = [big.tile([128, L], bf16, name=f"F{c}") for c in range(4)]
            W4 = []
            w_engs = [nc.sync, nc.scalar, nc.sync, nc.scalar]
            for c in range(4):
                w = big.tile([128, 384], bf16, name=f"W{c}")
                w_engs[c].dma_start(out=w, in_=wt_p[c * 128:(c + 1) * 128, :])
                W4.append(w)
            dma_engs = [nc.sync, nc.scalar, nc.scalar, nc.sync]
            def fblk(c, t):
                b = c * 4 + t
                return fmap_p[b * 128:(b + 1) * 128, :]

            for h in range(2):
                for c in range(4):
                    dma_engs[c].dma_start(
                        out=F4[c][:, h * 512:(h + 1) * 512],
                        in_=fblk(c, 0)[:, h * 512:(h + 1) * 512])
            for t in range(1, 4):
                for c in range(4):
                    dma_engs[c].dma_start(
                        out=F4[c][:, t * 1024:(t + 1) * 1024], in_=fblk(c, t))
            for k in range(4):
                (nc.sync if k % 2 == 0 else nc.scalar).dma_start(
                    out=ind_sb[:, k * 1024:(k + 1) * 1024],
                    in_=ind_p[k * 128:(k + 1) * 128, :])
            nc.gpsimd.dma_start(out=relh_sb, in_=relh_p[:, :])
            nc.gpsimd.dma_start(out=relw_sb, in_=relw_p[:, :])
            nc.gpsimd.dma_start(out=onesh_sb, in_=onesh_p[:, :])
            nc.gpsimd.dma_start(out=ident_sb, in_=ident_p[:, :])
            nc.gpsimd.dma_start(out=bias4_sb, in_=bias4_p[:, :])

            QT = big.tile([128, NQ], bf16, name="QT")
            KT = big.tile([128, L], bf16, name="KT")
            VTt = big.tile([128, L], bf16, name="VTt")
            Vn = big.tile([128, L], bf16, name="Vn")
            BT = big.tile([128, NQ], bf16, name="BT")
            bh_stage = big.tile([96, NQ], f32, name="bh_stage")
            bw_stage = big.tile([127, NQ], f32, name="bw_stage")
            bh2 = big.tile([64, NQ], f32, name="bh2")
            bw2 = big.tile([64, NQ], f32, name="bw2")

            # ---- PE warmup: dummy matmuls on the memset tile fill the early
            # DMA wait and open the HAM clock-gate before real matmuls ----
            with tc.tile_pool(name="psW", bufs=1, space="PSUM") as psW:
                wps = psW.tile([128, 512], f32, name="warm_ps", tag="warm")
                for _ in range(8):
                    nc.tensor.matmul(wps, warm_sb[:, 0:128], warm_sb,
                                     start=True, stop=True)

            # ---- phase A+B: qkv projection pipelined with fmap stripe DMAs;
            # bias built mid-stream once QT is complete ----
            with tc.tile_pool(name="psA", bufs=2, space="PSUM") as psA:
                def qkv_group(dst, col0, t, eng):
                    ps = psA.tile([128, 1024], f32, name="qkv_ps", tag="qkv", bufs=2)
                    for c in range(4):
                        for h in range(2):
                            nc.tensor.matmul(
                                ps[:, h * 512:(h + 1) * 512],
                                W4[c][:, col0:col0 + 128],
                                F4[c][:, t * 1024 + h * 512: t * 1024 + (h + 1) * 512],
                                start=(c == 0), stop=(c == 3))
                    if eng == "act":
                        nc.scalar.copy(dst[:, t * 1024:(t + 1) * 1024], ps)
                    else:
                        nc.vector.tensor_copy(dst[:, t * 1024:(t + 1) * 1024], ps)

                def bias_matmuls():
                    # batched rel-logit matmuls: every shift-window at once
                    # into PSUM, evacuated whole to SBUF staging, staged out
                    # to DRAM scratch (window order reversed via host-side
                    # reversed rel tables so the shear strides are positive).
                    # bh_stage[m, q] = sum_d relh_rev[d, m] * QT[d, q]
                    # bw_stage[m, (w*32+i)] = sum_d relw_rev[d, m] * QT[d, q(i,w)]
                    qt_w = QT.rearrange("d (i w) -> d w i", w=64)
                    for blk in range(4):
                        bh_ps = psA.tile([96, 512], f32, name="bh_ps", tag="bias", bufs=2)
                        nc.tensor.matmul(bh_ps, relh_sb,
                                         QT[:, blk * 512:(blk + 1) * 512],
                                         start=True, stop=True)
                        nc.vector.tensor_copy(
                            bh_stage[:, blk * 512:(blk + 1) * 512], bh_ps)
                        nc.sync.dma_start(
                            out=bh_d[:, blk * 512:(blk + 1) * 512],
                            in_=bh_stage[:, blk * 512:(blk + 1) * 512])
                        bw_ps = psA.tile([127, 512], f32, name="bw_ps", tag="bias", bufs=2)
                        nc.tensor.matmul(bw_ps, relw_sb,
                                         qt_w[:, blk * 16:(blk + 1) * 16, :],
                                         start=True, stop=True)
                        nc.scalar.copy(
                            bw_stage[:, blk * 512:(blk + 1) * 512], bw_ps)
                        nc.scalar.dma_start(
                            out=bw_d[:, blk * 512:(blk + 1) * 512],
                            in_=bw_stage[:, blk * 512:(blk + 1) * 512])

                def v_transposes(s0, s1):
                    # PE identity-transpose (DMA transposes monopolize the
                    # shared SDMA engine pool and crush all other DMA
                    # bandwidth); PSUM bf16 out, evacuated by ACT/DVE.
                    for s in range(s0, s1):
                        vt_ps = psA.tile([128, 128], bf16, name="vt_ps",
                                         tag="vt", bufs=2)
                        nc.tensor.transpose(
                            vt_ps, VTt[:, s * 128:(s + 1) * 128], ident_sb)
                        dst = Vn[:, s * 128:(s + 1) * 128]
                        if s % 2 == 0:
                            nc.scalar.copy(dst, vt_ps)
                        else:
                            nc.vector.tensor_copy(dst, vt_ps)

                # both Q stripes first so the bias pipeline (matmuls -> PSUM
                # evac -> DRAM stage-out -> slow strided shear read-back)
                # launches early and overlaps the remaining K/V projection.
                qkv_group(QT, 0, 0, "dve")
                qkv_group(KT, 128, 0, "act")
                qkv_group(QT, 0, 1, "dve")
                qkv_group(VTt, 256, 0, "act")
                bias_matmuls()
                # single strided shear reads: with reversed window order,
                # src flat index is (1+p+rr)*2048 + rr*64 + w for bh and
                # (c+w)*2048 + w*32 + i for bw — all positive strides,
                # contiguous innermost runs, one DMA per queue.
                nc.sync.dma_start(
                    out=bh2.rearrange("p (r w) -> p r w", r=32, w=64),
                    in_=RawAP(bh_d, NQ, [[NQ, 64], [NQ + 64, 32], [1, 64]]))
                nc.scalar.dma_start(
                    out=bw2.rearrange("p (w i) -> p w i", w=64, i=32),
                    in_=RawAP(bw_d, 0, [[NQ, 64], [NQ + 32, 64], [1, 32]]))
                nc.vector.tensor_copy(BT[0:64, :], bh2)
                nc.vector.tensor_copy(
                    BT[64:128, :].rearrange("p (i w) -> p i w", i=32, w=64),
                    bw2.rearrange("p (w i) -> p i w", w=64, i=32))
                v_transposes(0, 8)
                for t in range(1, 4):
                    qkv_group(KT, 128, t, "act")
                    qkv_group(VTt, 256, t, "act")
                    v_transposes(t * 8, (t + 1) * 8)

            # ---- phase C: attention main loop ----
            with tc.tile_pool(name="psC", bufs=1, space="PSUM") as psC:
                for qb in range(2):
                    q0 = qb * QB
                    acc = work.tile([128, QB], fp16, name="acc", tag="acc", bufs=2)
                    acc2 = work.tile([128, QB], fp16, name="acc2", tag="acc2", bufs=2)
                    outT = psC.tile([128, QB], f32, name="outT", tag="out", bufs=2)
                    for kc in range(32):
                        sim = psC.tile([128, QB], f32, name="sim", tag="sim", bufs=2)
                        for h in range(2):
                            sl = slice(q0 + h * 512, q0 + (h + 1) * 512)
                            po = sim[:, h * 512:(h + 1) * 512]
                            nc.tensor.matmul(
                                po, KT[:, kc * 128:(kc + 1) * 128], QT[:, sl],
                                start=True, stop=False)
                            nc.tensor.matmul(
                                po, ind_sb[:, kc * 128:(kc + 1) * 128], BT[:, sl],
                                start=False, stop=True)
                        expT = work.tile([128, QB], fp16, name="expT", tag="exp", bufs=8)
                        nc.scalar.activation(expT, sim, EXPF, bias=bias4_sb[:, 0:1], scale=SCALE)
                        if kc == 31:
                            last_expT = expT  # reduced directly by the rowsum matmul
                        else:
                            a = acc if kc < 16 else acc2
                            if kc in (0, 16):
                                nc.vector.tensor_copy(a, expT)
                            else:
                                nc.vector.tensor_add(a, a, expT)
                        for h in range(2):
                            nc.tensor.matmul(
                                outT[:, h * 512:(h + 1) * 512],
                                Vn[:, kc * 128:(kc + 1) * 128],
                                expT[:, h * 512:(h + 1) * 512],
                                start=(kc == 0), stop=(kc == 31))

                    # normalize in pipelined 512-wide halves: rowsum
                    # (ones-matmul partition reduce) -> broadcast (K=1 outer
                    # product) -> approx reciprocal -> scale -> store
                    for hh in range(2):
                        sl = slice(hh * 512, (hh + 1) * 512)
                        rs_ps = psC.tile([1, 512], f32, name="rs_ps", tag="sim", bufs=2)
                        nc.tensor.matmul(rs_ps, onesh_sb[:, 0:1], acc[:, sl],
                                         start=True, stop=False)
                        nc.tensor.matmul(rs_ps, onesh_sb[:, 0:1], acc2[:, sl],
                                         start=False, stop=False)
                        nc.tensor.matmul(rs_ps, onesh_sb[:, 0:1], last_expT[:, sl],
                                         start=False, stop=True)
                        rs_row = work.tile([1, 512], fp16, name="rs_row", tag="rsrow", bufs=2)
                        nc.scalar.copy(rs_row, rs_ps)
                        bc_ps = psC.tile([128, 512], f32, name="bc_ps", tag="sim", bufs=2)
                        nc.tensor.matmul(bc_ps, onesh_sb[0:1, :], rs_row,
                                         start=True, stop=True)
                        rec_sb = work.tile([128, 512], f32, name="rec_sb", tag="bc", bufs=2)
                        nc.vector.reciprocal_approx_fast(out=rec_sb, in_=bc_ps)
                        out_sb = work.tile([128, 512], f32, name="out_sb", tag="osb", bufs=2)
                        nc.vector.tensor_mul(out_sb, outT[:, sl], rec_sb)
                        eng = nc.sync if hh == 0 else nc.scalar
                        eng.dma_start(out=out_p[:, q0 + hh * 512:q0 + (hh + 1) * 512],
                                      in_=out_sb)

    nc.finalize()
    return nc


def _prep_core_inputs(fmap, w_qkv, rel_height, rel_width, core):
    bf = ml_dtypes.bfloat16
    h, half = core // 2, core % 2
    q0 = half * NQ
    perm = (np.arange(L) + q0) % L
    fmap_flat = fmap.reshape(C, L)
    fmap_core = np.ascontiguousarray(fmap_flat[:, perm]).astype(bf)
    rows = np.r_[h * 128:(h + 1) * 128,
                 512 + h * 128:512 + (h + 1) * 128,
                 1024 + h * 128:1024 + (h + 1) * 128]
    wt = np.ascontiguousarray(w_qkv[rows].T).astype(bf)
    relhT = rel_height.T  # (128, 127)
    a = 32 * (1 - half)
    relh_slab = np.zeros((128, 96), np.float32)
    relh_slab[:, :95] = relhT[:, a:a + 95]
    relh_slab = relh_slab[:, ::-1]  # reversed window order (shear strides > 0)
    relw = np.ascontiguousarray(rel_width.T[:, ::-1]).astype(bf)
    j = np.arange(L)
    ind = np.zeros((128, L), np.float32)
    ind[63 - (j // 64 + 32 * half) % 64, j] = 1.0
    ind[64 + 63 - (j % 64), j] = 1.0
    fmap_blocks = np.ascontiguousarray(
        fmap_core.reshape(4, 128, 4, 1024).transpose(0, 2, 1, 3).reshape(16 * 128, 1024))
    ind_blocks = np.ascontiguousarray(
        ind.reshape(128, 4, 1024).transpose(1, 0, 2).reshape(4 * 128, 1024))

    return {
        "fmapc": fmap_blocks,
        "wt": wt,
        "relh": relh_slab.astype(bf),
        "relw": relw,
        "ind": ind_blocks.astype(bf),
        "onesh": np.ones((128, 128), np.float16),
        "ident": np.eye(128, dtype=np.float32).astype(bf),
        "bias4": np.full((128, 1), -4.0, np.float32),
    }


def _install_trace_hook():
    """Register the axon NTFF profiling hook (missing antenv.axon_hooks shim)
    and neuter the artifact upload so tracing works in this sandbox."""
    import sys
    import types
    import concourse.bass_utils as bu
    bu.upload_artifacts = lambda d: d
    try:
        from antenv import axon_hooks  # noqa: F401
        return
    except ImportError:
        pass
    import antenv
    mod = types.ModuleType("antenv.axon_hooks")
    mod._hook = None
    def set_axon_ntff_profile_hook(h):
        mod._hook = h
    def get_axon_ntff_profile_hook():
        return mod._hook
    mod.set_axon_ntff_profile_hook = set_axon_ntff_profile_hook
    mod.get_axon_ntff_profile_hook = get_axon_ntff_profile_hook
    sys.modules["antenv.axon_hooks"] = mod
    antenv.axon_hooks = mod
    try:
        from trn_agent_boot.trn_boot import _ntff_profile_via_ctypes
        h = _ntff_profile_via_ctypes("/opt/axon/libaxon_pjrt.so")
        if h is not None:
            mod._hook = h
    except Exception as e:
        print(f"trace hook install failed: {e}")


def kernel(fmap, w_qkv, rel_height, rel_width, _trace=False):
    global _GRAPH
    from concourse.bass_utils import run_bass_kernel_spmd

    fmap = np.asarray(fmap, dtype=np.float32)
    w_qkv = np.asarray(w_qkv, dtype=np.float32)
    rel_height = np.asarray(rel_height, dtype=np.float32)
    rel_width = np.asarray(rel_width, dtype=np.float32)

    if _GRAPH is None:
        _GRAPH = _build_graph()
    nc = _GRAPH

    in_maps = [_prep_core_inputs(fmap, w_qkv, rel_height, rel_width, c)
               for c in range(NCORES)]
    kw = {}
    if _trace:
        _install_trace_hook()
        import os
        os.makedirs("/tmp/ktrace", exist_ok=True)
        import tempfile
        kw = dict(tmpdir=tempfile.mkdtemp(dir="/tmp/ktrace"))
    res = None
    last_err = None
    for attempt in range(3):
        try:
            res = run_bass_kernel_spmd(nc, in_maps, core_ids=list(range(NCORES)),
                                       trace=_trace, **kw)
            break
        except Exception as e:  # transient PJRT/tunnel hiccups
            last_err = e
    if res is None:
        raise last_err
    out_full = np.zeros((C, L), np.float32)
    for c in range(NCORES):
        h, half = c // 2, c % 2
        out_full[h * 128:(h + 1) * 128, half * NQ:(half + 1) * NQ] = \
            np.asarray(res.results[c]["out"])
    if _trace:
        kernel._last_exec_time_ns = res.exec_time_ns
        kernel._last_profile = res.profile_json
    return out_full.reshape(1, C, H, W)
